# revision 9
# baseline (speedup 1.0000x reference)
"""Trainium2 Bass kernel for a single-step DecoderRNN (reformat + embed+relu +
LSTM cell + vocab output projection), sharded over 8 NeuronCores.

Sharding: each core m owns hidden indices [128m, 128m+128). It computes its
slice of the LSTM state update and a partial [1, V] logits contribution from
its 128 columns of W_out (contraction dim); the host sums the 8 partials.
No cross-core communication on device.

Host-side folding: h = hidden @ W_ref.T + b_ref only feeds the gates through
W_hh, so W_hh @ W_ref is precomputed on host and the gate matvec contracts
directly against concat(relu(emb[tok]), hidden).

Compute strategy (v4):
- LSTM gates + cell reformat: PE streaming matmuls (stationary = z column
  tile [128,1], moving = fp32 weight tile), outputs as [1,512]/[1,128] rows;
  elementwise LSTM on 1-lane row slices (tiny).
- W_out partial logits split across engines so both hide under the DMA
  stream:
  * vocab [0, VP_PE) on PE with a bf16 hi/lo weight split: W = W_hi + W_lo
    (two bf16 planes, same HBM bytes as fp32), h = h_hi + h_lo likewise;
    logits = h_hi*W_hi + h_lo*W_hi + h_hi*W_lo accumulated in fp32 PSUM
    (dropped h_lo*W_lo term is ~2^-32 relative). bf16 streams at 1 cyc/row
    with N=1024 vs fp32's 4 cyc/row at N<=512.
  * vocab [VP_PE, VPAD) on DVE: natural layout, broadcast-h tensor_mul +
    segmented tensor_reduce in exact fp32.
- All bulk weight DMAs go on the sync HWDGE ring in priority order (LSTM
  weights first, then W chunks interleaved PE/DVE); small latency-critical
  DMAs (h round-trip/broadcast, psum evacuations) use the GpSimd SWDGE ring
  so they never queue behind multi-MB transfers.
"""
import numpy as np
import ml_dtypes

H = 1024
V = 50257
N_CORES = 8
VPAD = 51200          # 400 vocab tiles of 128
KZ = 2176             # gate contraction: 1024 (x) + 1028 (hidden) padded
NT_Z = KZ // 128      # 17
KC = 1152             # cell contraction: 1028 padded
NT_C = KC // 128      # 9
VP_PE = 32768         # vocab handled by TensorE (bf16 hi/lo split)
VP_DVE = VPAD - VP_PE # 18432 vocab on VectorE
PCHUNK = 4096         # PE vocab per weight chunk (8 chunks, hi+lo each)
N_WT = VP_PE // PCHUNK    # 8
N_PAIR = VP_PE // 1024    # 32 psum tiles
VCHUNK = VP_DVE // 4      # 4608 vocab per DVE chunk
N_WN = VP_DVE // VCHUNK   # 4
TPC = VCHUNK // 128       # 36 tiles per DVE chunk
NVT_DVE = VP_DVE // 128   # 144

_cache = {}


def _build_bass():
    import concourse.bacc as bacc
    import concourse.bass as bass
    from concourse import mybir, tile

    f32 = mybir.dt.float32
    bf16 = mybir.dt.bfloat16
    AF = mybir.ActivationFunctionType
    ALU = mybir.AluOpType

    nc = bacc.Bacc("TRN2", target_bir_lowering=False, debug=False,
                   num_devices=N_CORES)

    z_d = nc.dram_tensor("z", [128, NT_Z], f32, kind="ExternalInput")
    cz_d = nc.dram_tensor("cz", [128, NT_C], f32, kind="ExternalInput")
    g_d = nc.dram_tensor("g_w", [KZ, 512], f32, kind="ExternalInput")
    ac_d = nc.dram_tensor("ac_w", [KC, 128], f32, kind="ExternalInput")
    bias_d = nc.dram_tensor("bias", [1, 512], f32, kind="ExternalInput")
    bref_d = nc.dram_tensor("bref", [1, 128], f32, kind="ExternalInput")
    wth_d = nc.dram_tensor("wt_hi", [128, VP_PE], bf16, kind="ExternalInput")
    wtl_d = nc.dram_tensor("wt_lo", [128, VP_PE], bf16, kind="ExternalInput")
    wn_d = nc.dram_tensor("wn", [VP_DVE, 128], f32, kind="ExternalInput")
    oh_d = nc.dram_tensor("out_h", [1, 128], f32, kind="ExternalOutput")
    oc_d = nc.dram_tensor("out_c", [1, 128], f32, kind="ExternalOutput")
    ope_d = nc.dram_tensor("out_pe", [N_PAIR, 1024], f32, kind="ExternalOutput")
    opd_d = nc.dram_tensor("out_dve", [128, NVT_DVE], f32, kind="ExternalOutput")

    with tile.TileContext(nc) as tc:
        with (
            tc.tile_pool(name="const", bufs=1) as cpool,
            tc.tile_pool(name="wtp", bufs=4) as wtpool,
            tc.tile_pool(name="wnp", bufs=2) as wnpool,
            tc.tile_pool(name="scr", bufs=1) as spool,
            tc.tile_pool(name="row", bufs=1) as rpool,
            tc.tile_pool(name="evac", bufs=3) as epool,
            tc.tile_pool(name="ps", bufs=2, space=bass.MemorySpace.PSUM) as pspool,
            tc.tile_pool(name="psl", bufs=1, space=bass.MemorySpace.PSUM) as pslpool,
            tc.tile_pool(name="dram", bufs=1, space="DRAM") as dpool,
        ):
            # ---- LSTM inputs (first in the sync FIFO = highest priority) ----
            z_sb = cpool.tile([128, NT_Z], f32)
            nc.sync.dma_start(z_sb[:], z_d.ap())
            nc.scalar.activation(z_sb[:, 0:8], z_sb[:, 0:8], AF.Relu)
            cz_sb = cpool.tile([128, NT_C], f32)
            nc.sync.dma_start(cz_sb[:], cz_d.ap())
            ac_sb = cpool.tile([128, NT_C, 128], f32)
            nc.sync.dma_start(ac_sb[:], ac_d.ap().rearrange("(t p) j -> p t j", p=128))
            bias_sb = cpool.tile([1, 512], f32)
            nc.sync.dma_start(bias_sb[:], bias_d.ap())
            bref_sb = cpool.tile([1, 128], f32)
            nc.sync.dma_start(bref_sb[:], bref_d.ap())
            # gate weights in 4 slabs so matmuls pipeline with the transfer
            g_sb = cpool.tile([128, NT_Z, 512], f32)
            slabs = [(0, 5), (5, 9), (9, 13), (13, 17)]
            for a, b in slabs:
                nc.sync.dma_start(
                    g_sb[:, a:b, :],
                    g_d.ap()[a * 128:b * 128, :].rearrange("(t p) j -> p t j", p=128))

            # ---- gates: [1,512] = sum_t z_t^T @ G_t ----
            psum_g = pslpool.tile([1, 512], f32, tag="pg")
            for t in range(NT_Z):
                nc.tensor.matmul(psum_g[:], z_sb[:, t:t + 1], g_sb[:, t, :],
                                 start=(t == 0), stop=(t == NT_Z - 1))
            # ---- cell reformat: [1,128] ----
            psum_c = pslpool.tile([1, 128], f32, tag="pc")
            for t in range(NT_C):
                nc.tensor.matmul(psum_c[:], cz_sb[:, t:t + 1], ac_sb[:, t, :],
                                 start=(t == 0), stop=(t == NT_C - 1))

            # ---- elementwise LSTM on row layout ----
            gr = rpool.tile([1, 512], f32, tag="gr")
            nc.vector.tensor_add(gr[:], psum_g[:], bias_sb[:])
            si = rpool.tile([1, 128], f32, tag="si")
            nc.scalar.activation(si[:], gr[:, 0:128], AF.Sigmoid)
            sf = rpool.tile([1, 128], f32, tag="sf")
            nc.scalar.activation(sf[:], gr[:, 128:256], AF.Sigmoid)
            tg = rpool.tile([1, 128], f32, tag="tg")
            nc.scalar.activation(tg[:], gr[:, 256:384], AF.Tanh)
            so = rpool.tile([1, 128], f32, tag="so")
            nc.scalar.activation(so[:], gr[:, 384:512], AF.Sigmoid)
            c_in = rpool.tile([1, 128], f32, tag="ci")
            nc.vector.tensor_add(c_in[:], psum_c[:], bref_sb[:])
            m1 = rpool.tile([1, 128], f32, tag="m1")
            nc.vector.tensor_mul(m1[:], sf[:], c_in[:])
            c_new = rpool.tile([1, 128], f32, tag="cn")
            nc.vector.tensor_mul(c_new[:], si[:], tg[:])
            nc.vector.tensor_add(c_new[:], c_new[:], m1[:])
            tcn = rpool.tile([1, 128], f32, tag="tc")
            nc.scalar.activation(tcn[:], c_new[:], AF.Tanh)
            h_row = rpool.tile([1, 128], f32, tag="hr")
            nc.vector.tensor_mul(h_row[:], so[:], tcn[:])

            nc.gpsimd.dma_start(oh_d.ap(), h_row[:])
            nc.gpsimd.dma_start(oc_d.ap(), c_new[:])

            # h round trip: column view for PE, partition-broadcast for DVE
            h_rt = dpool.tile([1, 128], f32)
            nc.gpsimd.dma_start(h_rt[:], h_row[:])
            h_col = cpool.tile([128, 1], f32)
            nc.gpsimd.dma_start(h_col[:], h_rt[:].rearrange("one p -> p one"))
            hb_sb = cpool.tile([128, 128], f32)
            nc.gpsimd.dma_start(hb_sb[:], h_rt[:].broadcast_to([128, 128]))
            hb_bc = hb_sb[:].rearrange("p (c k) -> p c k", c=1).broadcast_to(
                [128, TPC, 128])

            # bf16 hi/lo split of h for the PE path
            h_hi = cpool.tile([128, 1], bf16)
            nc.vector.tensor_copy(h_hi[:], h_col[:])
            h_hi_f = cpool.tile([128, 1], f32)
            nc.vector.tensor_copy(h_hi_f[:], h_hi[:])
            h_lo_f = cpool.tile([128, 1], f32)
            nc.vector.tensor_sub(h_lo_f[:], h_col[:], h_hi_f[:])
            h_lo = cpool.tile([128, 1], bf16)
            nc.vector.tensor_copy(h_lo[:], h_lo_f[:])

            # ---- W_out: interleave PE chunks and DVE chunks on the sync ring ----
            out_dve = cpool.tile([128, NVT_DVE], f32)
            wt_tiles = []
            for ch in range(N_WT):
                whi = wtpool.tile([128, PCHUNK], bf16, tag="whi")
                nc.sync.dma_start(
                    whi[:], wth_d.ap()[:, ch * PCHUNK:(ch + 1) * PCHUNK])
                wlo = wtpool.tile([128, PCHUNK], bf16, tag="wlo")
                nc.sync.dma_start(
                    wlo[:], wtl_d.ap()[:, ch * PCHUNK:(ch + 1) * PCHUNK])
                wt_tiles.append((whi, wlo))
                if ch < N_WN:
                    wn_sb = wnpool.tile([128, TPC, 128], f32)
                    nc.sync.dma_start(
                        wn_sb[:],
                        wn_d.ap()[ch * VCHUNK:(ch + 1) * VCHUNK, :]
                        .rearrange("(c p) k -> p c k", p=128))
                    mscr = spool.tile([128, VCHUNK], f32, tag="scr")
                    m3 = mscr[:].rearrange("p (c k) -> p c k", k=128)
                    nc.vector.tensor_mul(m3, wn_sb[:], hb_bc)
                    nc.vector.tensor_reduce(
                        out_dve[:, ch * TPC:(ch + 1) * TPC], m3,
                        mybir.AxisListType.X, ALU.add)

            pairs_per_chunk = PCHUNK // 1024  # 4
            for pair in range(N_PAIR):
                whi, wlo = wt_tiles[pair // pairs_per_chunk]
                base = (pair % pairs_per_chunk) * 1024
                ps = pspool.tile([1, 1024], f32, tag="wps")
                for half in range(2):
                    b = base + half * 512
                    po = ps[:, half * 512:(half + 1) * 512]
                    nc.tensor.matmul(po, h_hi[:], whi[:, b:b + 512],
                                     start=True, stop=False)
                    nc.tensor.matmul(po, h_lo[:], whi[:, b:b + 512],
                                     start=False, stop=False,
                                     skip_group_check=True)
                    nc.tensor.matmul(po, h_hi[:], wlo[:, b:b + 512],
                                     start=False, stop=True,
                                     skip_group_check=True)
                row = epool.tile([1, 1024], f32, tag="evac")
                nc.scalar.activation(row[:], ps[:], AF.Copy)
                nc.gpsimd.dma_start(ope_d.ap()[pair:pair + 1, :], row[:])

            nc.sync.dma_start(opd_d.ap(), out_dve[:])

    nc.finalize()
    return nc


def _prep_in_maps(input, hidden, cell, W_ref, b_ref, emb, W_ih, W_hh, b_ih,
                  b_hh, W_out, b_out):
    tok = int(np.asarray(input).reshape(-1)[0])
    x_row = np.asarray(emb[tok], dtype=np.float32).reshape(H)
    hidden_flat = np.asarray(hidden, dtype=np.float32).reshape(-1)
    cell_flat = np.asarray(cell, dtype=np.float32).reshape(-1)
    W_ref = np.asarray(W_ref, dtype=np.float32)
    b_ref = np.asarray(b_ref, dtype=np.float32)
    W_ih = np.asarray(W_ih, dtype=np.float32)
    W_hh = np.asarray(W_hh, dtype=np.float32)
    b_ih = np.asarray(b_ih, dtype=np.float32)
    b_hh = np.asarray(b_hh, dtype=np.float32)
    W_out = np.asarray(W_out, dtype=np.float32)
    KH = W_ref.shape[1]  # 1028

    W_hh_eff = W_hh @ W_ref                      # [4H, KH]
    bias_eff = b_ih + b_hh + W_hh @ b_ref        # [4H]

    z_raw = np.zeros(KZ, np.float32)
    z_raw[:H] = x_row
    z_raw[H:H + KH] = hidden_flat
    z_host = np.ascontiguousarray(z_raw.reshape(NT_Z, 128).T)     # [128,17]

    cz_raw = np.zeros(KC, np.float32)
    cz_raw[:KH] = cell_flat
    cz_host = np.ascontiguousarray(cz_raw.reshape(NT_C, 128).T)   # [128,9]

    WT = np.ascontiguousarray(W_out.T)           # [H, V]

    in_maps = []
    for m in range(N_CORES):
        sl = slice(128 * m, 128 * (m + 1))
        rows = (np.arange(4)[:, None] * H + 128 * m + np.arange(128)[None, :]
                ).reshape(-1)                    # [512]
        G = np.zeros((KZ, 512), np.float32)
        G[:H] = W_ih[rows].T
        G[H:H + KH] = W_hh_eff[rows].T
        Ac = np.zeros((KC, 128), np.float32)
        Ac[:KH] = W_ref[sl].T
        bias_m = np.ascontiguousarray(bias_eff[rows].reshape(1, 512))
        bref_m = np.ascontiguousarray(b_ref[sl].reshape(1, 128))
        Wt = np.ascontiguousarray(WT[sl, :VP_PE])           # [128, VP_PE]
        Wt_hi = Wt.astype(ml_dtypes.bfloat16)
        Wt_lo = (Wt - Wt_hi.astype(np.float32)).astype(ml_dtypes.bfloat16)
        Wn = np.zeros((VP_DVE, 128), np.float32)
        Wn[:V - VP_PE] = W_out[VP_PE:, sl]
        in_maps.append({
            "z": z_host, "cz": cz_host, "g_w": G, "ac_w": Ac,
            "bias": bias_m, "bref": bref_m,
            "wt_hi": Wt_hi, "wt_lo": Wt_lo, "wn": Wn,
        })
    return in_maps


def _assemble(results, b_out):
    b_out = np.asarray(b_out, dtype=np.float32)
    pe = np.zeros(VP_PE, np.float32)
    dve = np.zeros((128, NVT_DVE), np.float32)
    h_parts, c_parts = [], []
    for r in results:
        pe += r["out_pe"].reshape(-1)
        dve += r["out_dve"]
        h_parts.append(r["out_h"].reshape(128))
        c_parts.append(r["out_c"].reshape(128))
    logits = np.empty(VPAD, np.float32)
    logits[:VP_PE] = pe
    logits[VP_PE:] = np.ascontiguousarray(dve.T).reshape(-1)
    logits = logits[:V] + b_out
    output = logits.reshape(1, V).astype(np.float32)
    h_new = np.concatenate(h_parts).reshape(1, 1, H).astype(np.float32)
    c_new = np.concatenate(c_parts).reshape(1, 1, H).astype(np.float32)
    return output, h_new, c_new


def _kernel_impl(inputs, trace=False, trace_cores=None):
    from concourse.bass_utils import run_bass_kernel_spmd

    if "nc" not in _cache:
        _cache["nc"] = _build_bass()
    nc = _cache["nc"]
    in_maps = _prep_in_maps(**inputs)
    res = run_bass_kernel_spmd(nc, in_maps, list(range(N_CORES)),
                               trace=trace, trace_cores=trace_cores)
    outs = _assemble(res.results, inputs["b_out"])
    return outs, res


def kernel(**inputs):
    outs, _ = _kernel_impl(inputs, trace=False)
    return outs


# revision 13
# speedup vs baseline: 1.0973x; 1.0973x over previous
"""Trainium2 Bass kernel for a single-step DecoderRNN (reformat + embed+relu +
LSTM cell + vocab output projection), sharded over 8 NeuronCores.

Sharding: each core m owns hidden indices [128m, 128m+128). It computes its
slice of the LSTM state update and a partial [1, V] logits contribution from
its 128 columns of W_out (contraction dim); the host sums the 8 partials.
No cross-core communication on device.

Host-side folding: h = hidden @ W_ref.T + b_ref only feeds the gates through
W_hh, so W_hh @ W_ref is precomputed on host and the gate matvec contracts
directly against concat(relu(emb[tok]), hidden).

Compute strategy (v4):
- LSTM gates + cell reformat: PE streaming matmuls (stationary = z column
  tile [128,1], moving = fp32 weight tile), outputs as [1,512]/[1,128] rows;
  elementwise LSTM on 1-lane row slices (tiny).
- W_out partial logits split across engines so both hide under the DMA
  stream:
  * vocab [0, VP_PE) on PE with a bf16 hi/lo weight split: W = W_hi + W_lo
    (two bf16 planes, same HBM bytes as fp32), h = h_hi + h_lo likewise;
    logits = h_hi*W_hi + h_lo*W_hi + h_hi*W_lo accumulated in fp32 PSUM
    (dropped h_lo*W_lo term is ~2^-32 relative). bf16 streams at 1 cyc/row
    with N=1024 vs fp32's 4 cyc/row at N<=512.
  * vocab [VP_PE, VPAD) on DVE: natural layout, broadcast-h tensor_mul +
    segmented tensor_reduce in exact fp32.
- All bulk weight DMAs go on the sync HWDGE ring in priority order (LSTM
  weights first, then W chunks interleaved PE/DVE); small latency-critical
  DMAs (h round-trip/broadcast, psum evacuations) use the GpSimd SWDGE ring
  so they never queue behind multi-MB transfers.
"""
import numpy as np
import ml_dtypes

H = 1024
V = 50257
N_CORES = 8
VPAD = 51200          # 400 vocab tiles of 128
KZ = 2176             # gate contraction: 1024 (x) + 1028 (hidden) padded
NT_Z = KZ // 128      # 17
KC = 1152             # cell contraction: 1028 padded
NT_C = KC // 128      # 9
VP_PE = 32768         # vocab handled by TensorE (bf16 hi/lo split)
VP_DVE = VPAD - VP_PE # 18432 vocab on VectorE
PCHUNK = 4096         # PE vocab per weight chunk (8 chunks, hi+lo each)
N_WT = VP_PE // PCHUNK    # 8
N_PAIR = VP_PE // 1024    # 32 psum tiles
VCHUNK = VP_DVE // 4      # 4608 vocab per DVE chunk
N_WN = VP_DVE // VCHUNK   # 4
TPC = VCHUNK // 128       # 36 tiles per DVE chunk
NVT_DVE = VP_DVE // 128   # 144

_cache = {}


def _build_bass():
    import concourse.bacc as bacc
    import concourse.bass as bass
    from concourse import mybir, tile

    f32 = mybir.dt.float32
    bf16 = mybir.dt.bfloat16
    AF = mybir.ActivationFunctionType
    ALU = mybir.AluOpType

    nc = bacc.Bacc("TRN2", target_bir_lowering=False, debug=False,
                   num_devices=N_CORES)

    z_d = nc.dram_tensor("z", [128, NT_Z], f32, kind="ExternalInput")
    cz_d = nc.dram_tensor("cz", [128, NT_C], f32, kind="ExternalInput")
    g_d = nc.dram_tensor("g_w", [KZ, 512], f32, kind="ExternalInput")
    ac_d = nc.dram_tensor("ac_w", [KC, 128], f32, kind="ExternalInput")
    bias_d = nc.dram_tensor("bias", [1, 512], f32, kind="ExternalInput")
    bref_d = nc.dram_tensor("bref", [1, 128], f32, kind="ExternalInput")
    wth_d = nc.dram_tensor("wt_hi", [128, VP_PE], bf16, kind="ExternalInput")
    wtl_d = nc.dram_tensor("wt_lo", [128, VP_PE], bf16, kind="ExternalInput")
    wn_d = nc.dram_tensor("wn", [VP_DVE, 128], f32, kind="ExternalInput")
    oh_d = nc.dram_tensor("out_h", [1, 128], f32, kind="ExternalOutput")
    oc_d = nc.dram_tensor("out_c", [1, 128], f32, kind="ExternalOutput")
    ope_d = nc.dram_tensor("out_pe", [N_PAIR, 1024], f32, kind="ExternalOutput")
    opd_d = nc.dram_tensor("out_dve", [128, NVT_DVE], f32, kind="ExternalOutput")

    with tile.TileContext(nc) as tc:
        with (
            tc.tile_pool(name="const", bufs=1) as cpool,
            tc.tile_pool(name="wtp", bufs=3) as wtpool,
            tc.tile_pool(name="wnp", bufs=2) as wnpool,
            tc.tile_pool(name="scr", bufs=1) as spool,
            tc.tile_pool(name="row", bufs=1) as rpool,
            tc.tile_pool(name="evac", bufs=3) as epool,
            tc.tile_pool(name="ps", bufs=2, space=bass.MemorySpace.PSUM) as pspool,
            tc.tile_pool(name="psl", bufs=1, space=bass.MemorySpace.PSUM) as pslpool,
        ):
            # ---- LSTM inputs: tiny ones on the SWDGE ring (cheap issue),
            # ---- bulk weights lead the sync HWDGE FIFO ----
            z_sb = cpool.tile([128, NT_Z], f32)
            nc.gpsimd.dma_start(z_sb[:], z_d.ap())
            nc.scalar.activation(z_sb[:, 0:8], z_sb[:, 0:8], AF.Relu)
            cz_sb = cpool.tile([128, NT_C], f32)
            nc.gpsimd.dma_start(cz_sb[:], cz_d.ap())
            bias_sb = cpool.tile([1, 512], f32)
            nc.gpsimd.dma_start(bias_sb[:], bias_d.ap())
            bref_sb = cpool.tile([1, 128], f32)
            nc.gpsimd.dma_start(bref_sb[:], bref_d.ap())
            ones_sb = cpool.tile([1, 128], f32)
            nc.gpsimd.memset(ones_sb[:], 1.0)
            ident_sb = cpool.tile([1, 1], f32)
            nc.gpsimd.memset(ident_sb[:], 1.0)
            ac_sb = cpool.tile([128, NT_C, 128], f32)
            nc.sync.dma_start(ac_sb[:], ac_d.ap().rearrange("(t p) j -> p t j", p=128))
            # gate weights in 4 slabs so matmuls pipeline with the transfer
            g_sb = cpool.tile([128, NT_Z, 512], f32)
            slabs = [(0, 5), (5, 9), (9, 13), (13, 17)]
            for a, b in slabs:
                nc.sync.dma_start(
                    g_sb[:, a:b, :],
                    g_d.ap()[a * 128:b * 128, :].rearrange("(t p) j -> p t j", p=128))

            # ---- cell reformat: [1,128] (ac lands first, keep PE busy early)
            psum_c = pslpool.tile([1, 128], f32, tag="pc")
            for t in range(NT_C):
                nc.tensor.matmul(psum_c[:], cz_sb[:, t:t + 1], ac_sb[:, t, :],
                                 start=(t == 0), stop=(t == NT_C - 1))
            # ---- gates: [1,512] = sum_t z_t^T @ G_t ----
            psum_g = pslpool.tile([1, 512], f32, tag="pg")
            for t in range(NT_Z):
                nc.tensor.matmul(psum_g[:], z_sb[:, t:t + 1], g_sb[:, t, :],
                                 start=(t == 0), stop=(t == NT_Z - 1))

            # ---- elementwise LSTM on row layout ----
            gr = rpool.tile([1, 512], f32, tag="gr")
            nc.vector.tensor_add(gr[:], psum_g[:], bias_sb[:])
            si = rpool.tile([1, 128], f32, tag="si")
            nc.scalar.activation(si[:], gr[:, 0:128], AF.Sigmoid)
            sf = rpool.tile([1, 128], f32, tag="sf")
            nc.scalar.activation(sf[:], gr[:, 128:256], AF.Sigmoid)
            tg = rpool.tile([1, 128], f32, tag="tg")
            nc.scalar.activation(tg[:], gr[:, 256:384], AF.Tanh)
            so = rpool.tile([1, 128], f32, tag="so")
            nc.scalar.activation(so[:], gr[:, 384:512], AF.Sigmoid)
            c_in = rpool.tile([1, 128], f32, tag="ci")
            nc.vector.tensor_add(c_in[:], psum_c[:], bref_sb[:])
            m1 = rpool.tile([1, 128], f32, tag="m1")
            nc.vector.tensor_mul(m1[:], sf[:], c_in[:])
            c_new = rpool.tile([1, 128], f32, tag="cn")
            nc.vector.tensor_mul(c_new[:], si[:], tg[:])
            nc.vector.tensor_add(c_new[:], c_new[:], m1[:])
            tcn = rpool.tile([1, 128], f32, tag="tc")
            nc.scalar.activation(tcn[:], c_new[:], AF.Tanh)
            h_row = rpool.tile([1, 128], f32, tag="hr")
            nc.vector.tensor_mul(h_row[:], so[:], tcn[:])

            nc.gpsimd.dma_start(oh_d.ap(), h_row[:])
            nc.gpsimd.dma_start(oc_d.ap(), c_new[:])

            # on-chip h relayout: PE transpose -> column, ones-matmul -> bcast
            ps_t = pslpool.tile([128, 1], f32, tag="pt")
            nc.tensor.transpose(ps_t[:], h_row[:], ident_sb[:])
            h_col = cpool.tile([128, 1], f32)
            nc.scalar.activation(h_col[:], ps_t[:], AF.Copy)
            ps_hb = pslpool.tile([128, 128], f32, tag="phb")
            nc.tensor.matmul(ps_hb[:], ones_sb[:], h_row[:],
                             start=True, stop=True)
            hb_sb = cpool.tile([128, 128], f32)
            nc.vector.tensor_copy(hb_sb[:], ps_hb[:])
            hb_bc = hb_sb[:].rearrange("p (c k) -> p c k", c=1).broadcast_to(
                [128, TPC, 128])

            # bf16 hi/lo split of h for the PE path
            h_hi = cpool.tile([128, 1], bf16)
            nc.vector.tensor_copy(h_hi[:], h_col[:])
            h_hi_f = cpool.tile([128, 1], f32)
            nc.vector.tensor_copy(h_hi_f[:], h_hi[:])
            h_lo_f = cpool.tile([128, 1], f32)
            nc.vector.tensor_sub(h_lo_f[:], h_col[:], h_hi_f[:])
            h_lo = cpool.tile([128, 1], bf16)
            nc.vector.tensor_copy(h_lo[:], h_lo_f[:])

            # ---- W_out: interleave PE chunks and DVE chunks on the sync ring ----
            out_dve = cpool.tile([128, NVT_DVE], f32)
            wt_tiles = []
            for ch in range(N_WT):
                whi = wtpool.tile([128, PCHUNK], bf16, tag="whi")
                nc.sync.dma_start(
                    whi[:], wth_d.ap()[:, ch * PCHUNK:(ch + 1) * PCHUNK])
                wlo = wtpool.tile([128, PCHUNK], bf16, tag="wlo")
                nc.sync.dma_start(
                    wlo[:], wtl_d.ap()[:, ch * PCHUNK:(ch + 1) * PCHUNK])
                wt_tiles.append((whi, wlo))
                if ch < N_WN:
                    wn_sb = wnpool.tile([128, TPC, 128], f32)
                    nc.sync.dma_start(
                        wn_sb[:],
                        wn_d.ap()[ch * VCHUNK:(ch + 1) * VCHUNK, :]
                        .rearrange("(c p) k -> p c k", p=128))
                    mscr = spool.tile([128, VCHUNK], f32, tag="scr")
                    m3 = mscr[:].rearrange("p (c k) -> p c k", k=128)
                    nc.vector.tensor_mul(m3, wn_sb[:], hb_bc)
                    nc.vector.tensor_reduce(
                        out_dve[:, ch * TPC:(ch + 1) * TPC], m3,
                        mybir.AxisListType.X, ALU.add)

            pairs_per_chunk = PCHUNK // 1024  # 4
            for pair in range(N_PAIR):
                whi, wlo = wt_tiles[pair // pairs_per_chunk]
                base = (pair % pairs_per_chunk) * 1024
                ps = pspool.tile([1, 1024], f32, tag="wps")
                for half in range(2):
                    b = base + half * 512
                    po = ps[:, half * 512:(half + 1) * 512]
                    nc.tensor.matmul(po, h_hi[:], whi[:, b:b + 512],
                                     start=True, stop=False)
                    nc.tensor.matmul(po, h_lo[:], whi[:, b:b + 512],
                                     start=False, stop=False,
                                     skip_group_check=True)
                    nc.tensor.matmul(po, h_hi[:], wlo[:, b:b + 512],
                                     start=False, stop=True,
                                     skip_group_check=True)
                row = epool.tile([1, 1024], f32, tag="evac")
                nc.scalar.activation(row[:], ps[:], AF.Copy)
                nc.gpsimd.dma_start(ope_d.ap()[pair:pair + 1, :], row[:])

            nc.sync.dma_start(opd_d.ap(), out_dve[:])

    nc.finalize()
    return nc


def _prep_in_maps(input, hidden, cell, W_ref, b_ref, emb, W_ih, W_hh, b_ih,
                  b_hh, W_out, b_out):
    tok = int(np.asarray(input).reshape(-1)[0])
    x_row = np.asarray(emb[tok], dtype=np.float32).reshape(H)
    hidden_flat = np.asarray(hidden, dtype=np.float32).reshape(-1)
    cell_flat = np.asarray(cell, dtype=np.float32).reshape(-1)
    W_ref = np.asarray(W_ref, dtype=np.float32)
    b_ref = np.asarray(b_ref, dtype=np.float32)
    W_ih = np.asarray(W_ih, dtype=np.float32)
    W_hh = np.asarray(W_hh, dtype=np.float32)
    b_ih = np.asarray(b_ih, dtype=np.float32)
    b_hh = np.asarray(b_hh, dtype=np.float32)
    W_out = np.asarray(W_out, dtype=np.float32)
    KH = W_ref.shape[1]  # 1028

    W_hh_eff = W_hh @ W_ref                      # [4H, KH]
    bias_eff = b_ih + b_hh + W_hh @ b_ref        # [4H]

    z_raw = np.zeros(KZ, np.float32)
    z_raw[:H] = x_row
    z_raw[H:H + KH] = hidden_flat
    z_host = np.ascontiguousarray(z_raw.reshape(NT_Z, 128).T)     # [128,17]

    cz_raw = np.zeros(KC, np.float32)
    cz_raw[:KH] = cell_flat
    cz_host = np.ascontiguousarray(cz_raw.reshape(NT_C, 128).T)   # [128,9]

    WT = np.ascontiguousarray(W_out.T)           # [H, V]

    in_maps = []
    for m in range(N_CORES):
        sl = slice(128 * m, 128 * (m + 1))
        rows = (np.arange(4)[:, None] * H + 128 * m + np.arange(128)[None, :]
                ).reshape(-1)                    # [512]
        G = np.zeros((KZ, 512), np.float32)
        G[:H] = W_ih[rows].T
        G[H:H + KH] = W_hh_eff[rows].T
        Ac = np.zeros((KC, 128), np.float32)
        Ac[:KH] = W_ref[sl].T
        bias_m = np.ascontiguousarray(bias_eff[rows].reshape(1, 512))
        bref_m = np.ascontiguousarray(b_ref[sl].reshape(1, 128))
        Wt = np.ascontiguousarray(WT[sl, :VP_PE])           # [128, VP_PE]
        Wt_hi = Wt.astype(ml_dtypes.bfloat16)
        Wt_lo = (Wt - Wt_hi.astype(np.float32)).astype(ml_dtypes.bfloat16)
        Wn = np.zeros((VP_DVE, 128), np.float32)
        Wn[:V - VP_PE] = W_out[VP_PE:, sl]
        in_maps.append({
            "z": z_host, "cz": cz_host, "g_w": G, "ac_w": Ac,
            "bias": bias_m, "bref": bref_m,
            "wt_hi": Wt_hi, "wt_lo": Wt_lo, "wn": Wn,
        })
    return in_maps


def _assemble(results, b_out):
    b_out = np.asarray(b_out, dtype=np.float32)
    pe = np.zeros(VP_PE, np.float32)
    dve = np.zeros((128, NVT_DVE), np.float32)
    h_parts, c_parts = [], []
    for r in results:
        pe += r["out_pe"].reshape(-1)
        dve += r["out_dve"]
        h_parts.append(r["out_h"].reshape(128))
        c_parts.append(r["out_c"].reshape(128))
    logits = np.empty(VPAD, np.float32)
    logits[:VP_PE] = pe
    logits[VP_PE:] = np.ascontiguousarray(dve.T).reshape(-1)
    logits = logits[:V] + b_out
    output = logits.reshape(1, V).astype(np.float32)
    h_new = np.concatenate(h_parts).reshape(1, 1, H).astype(np.float32)
    c_new = np.concatenate(c_parts).reshape(1, 1, H).astype(np.float32)
    return output, h_new, c_new


def _kernel_impl(inputs, trace=False, trace_cores=None):
    from concourse.bass_utils import run_bass_kernel_spmd

    if "nc" not in _cache:
        _cache["nc"] = _build_bass()
    nc = _cache["nc"]
    in_maps = _prep_in_maps(**inputs)
    res = run_bass_kernel_spmd(nc, in_maps, list(range(N_CORES)),
                               trace=trace, trace_cores=trace_cores)
    outs = _assemble(res.results, inputs["b_out"])
    return outs, res


def kernel(**inputs):
    outs, _ = _kernel_impl(inputs, trace=False)
    return outs
